# revision 21
# baseline (speedup 1.0000x reference)
"""Trainium2 Bass kernel for a 2-layer LSTM (H=64, T=256) + FC head.

Strategy: data-parallel over batch (8192 -> 1024 per core x 8 cores).
Within a core, hidden-major layout: states are [128, 512] tiles with
layer-1 state on partitions 0-63 and layer-2 state on partitions 64-127,
batch on the free dim (2 streams of 512 for pipelining). The two LSTM
layers run as a wavefront (layer 2 lags layer 1 by one timestep), so a
single set of [128, 512] elementwise ops covers both layers each round.

Per gate, the pre-activations of both layers come from one K=128 matmul
against the packed state [h1; h2] (the L1 half of the weight matrix has
zeros over the h2 rows) plus one K=14 accumulating matmul for the x
contribution. All matmuls run the full 128-wide array at partition base
0 -- explicit PE tile_position quadrants crash this toolchain.

Host-side prep is layout-only: x is transposed to [T*14, B] so the
kernel can DMA contiguous [14, batch] slices each timestep.
"""

import os
from contextlib import ExitStack

import numpy as np

import concourse.bass as bass
import concourse.tile as tile
from concourse import mybir
from concourse.bass import MemorySpace
from concourse.bass_utils import run_bass_kernel_spmd

F32 = mybir.dt.float32
F16 = mybir.dt.float16
BF16 = mybir.dt.bfloat16
AF = mybir.ActivationFunctionType


def _wait_cap(inst) -> int:
    """Max sync waits the walrus in this image encodes per instruction."""
    if isinstance(inst, mybir.InstEventSemaphore):
        return 2
    return 1


def _patch_tile():
    """The walrus in this image encodes at most one sync wait on most
    instruction structs, but the Tile scheduler attaches several. Split the
    extras onto wait-only NOPs inserted just before the instruction (same
    engine), both during lowering and at the kernel-tail drain."""
    import bass_rust

    if getattr(tile.TileContext, "_ant_wait_patched", False):
        return

    def _split_waits(self, inst):
        si = inst.sync_info
        if si is None:
            return
        cap = _wait_cap(inst)
        waits = list(si.on_wait)
        if len(waits) <= cap:
            return
        si.on_wait = waits[: cap - 1] + waits[-1:] if cap > 1 else waits[-1:]
        extras = waits[cap - 1 : -1] if cap > 1 else waits[:-1]
        for w in extras:
            nop = mybir.InstNoOp(
                name=self.nc.get_next_instruction_name(), ins=[], outs=[]
            )
            nop.engine = inst.engine
            nop.sync_info = bass_rust.SyncInfo(on_wait=[w], on_update=[])
            self.nc.register_instruction(nop, overwrite=True)
            self.nc.cur_bb.bb.add_instruction(nop)

    orig_add = tile.TileContext._add_instruction

    def _add_instruction(self, inst):
        if inst.engine != mybir.EngineType.Unassigned:
            _split_waits(self, inst)
        return orig_add(self, inst)

    from concourse.vector_clock import ScopedClock

    def _drain_and_barrier(self, tick_clock, wait_clock):
        nc = self.nc
        drain_inst = nc.sync.drain()
        wait_clock.add_sem_waits(
            drain_inst.ins, ScopedClock({None: tick_clock.global_clock})
        )
        si = drain_inst.ins.sync_info
        if si is not None and len(si.on_wait) > 1:
            waits = list(si.on_wait)
            si.on_wait = waits[-1:]
            for w in waits[:-1]:
                nop = nc.sync.nop(nofuse=True, hint="drain_wait_split")
                nop.ins.sync_info = bass_rust.SyncInfo(on_wait=[w], on_update=[])

        nc.all_engine_barrier()
        assert self.sems is not None
        popped = nc._tile_sem_poison_stack.pop()
        assert popped is self._sem_poison
        nc.clear_and_free_semaphores(list(self.sems.allocated().values()))
        nc.all_engine_barrier()

    tile.TileContext._add_instruction = _add_instruction
    tile.TileContext._drain_and_barrier = _drain_and_barrier
    tile.TileContext._ant_wait_patched = True


_patch_tile()

N_CORES = 8
B, T, I, H = 8192, 256, 14, 64
IB = I + 1                     # x rows per step incl. the ones row (PE bias)
XP = 128                       # x-matmul K padded to 128: mixing PE row
                               # tile-sizes (K=128 vs K<128) between matmuls
                               # drops the PE to ~40% throughput, so every
                               # matmul uses K=128. Rows IB:XP carry zero
                               # weights so their rhs values never matter
B_CORE = B // N_CORES          # 1024
N_STREAMS = 2
BS = B_CORE // N_STREAMS       # 512 batch elems per stream
GROUP = 8                      # rounds per x-DMA group

LAST_RESULTS = None  # BassKernelResults of the most recent kernel() call


def build(nc, t_steps=T, bs=BS, n_streams=N_STREAMS):
    """Emit the per-core program. Identical on all cores (SPMD)."""
    n_rounds = t_steps + 1
    n_groups = (n_rounds + GROUP - 1) // GROUP
    xt_rows = IB * n_groups * GROUP
    b_core = bs * n_streams

    x_d = nc.dram_tensor("xt", (xt_rows, b_core), F16, kind="ExternalInput")
    wh_d = nc.dram_tensor("wh", (128, 512), F16, kind="ExternalInput")
    wx_d = nc.dram_tensor("wx", (XP, 512), F16, kind="ExternalInput")
    wfc1_d = nc.dram_tensor("wfc1p", (128, H), F16, kind="ExternalInput")
    bfc1_d = nc.dram_tensor("bfc1", (H, 1), F32, kind="ExternalInput")
    wfc2_d = nc.dram_tensor("wfc2a", (H + 1, 2), F16, kind="ExternalInput")
    z0_d = nc.dram_tensor("z0", (128, bs), F16, kind="ExternalInput")
    y_d = nc.dram_tensor("y", (b_core, 2), F32, kind="ExternalOutput")

    with tile.TileContext(nc) as tc, ExitStack() as ctx:
        singles = ctx.enter_context(tc.tile_pool(name="singles", bufs=1))
        gp = ctx.enter_context(
            tc.tile_pool(name="gates", bufs=1, space=MemorySpace.PSUM)
        )
        sp = ctx.enter_context(tc.tile_pool(name="work", bufs=2))

        wh = singles.tile([128, 512], F16)      # [[Whh0_g, Wih1_g],[0, Whh1_g]] x4
        nc.sync.dma_start(out=wh, in_=wh_d[:, :])
        wx = singles.tile([XP, 512], F16)       # [[Wih0_g | 0]; [b0_g | b1_g]] x4
        nc.sync.dma_start(out=wx, in_=wx_d[:, :])
        wfc1 = singles.tile([128, H], F16)      # [0; Wfc1^T]
        nc.sync.dma_start(out=wfc1, in_=wfc1_d[:, :])
        bfc1 = singles.tile([H, 1], F32)
        nc.sync.dma_start(out=bfc1, in_=bfc1_d[:, :])
        wfc2 = singles.tile([H + 1, 2], F16)
        nc.sync.dma_start(out=wfc2, in_=wfc2_d[:, :])

        # --- persistent state: [h1; h2] and [c1; c2] per stream ---
        S = [
            singles.tile([128, bs], F16, tag=f"Sh{s}", name=f"Sh{s}")
            for s in range(n_streams)
        ]
        C = [
            singles.tile([128, bs], F16, tag=f"Cs{s}", name=f"Cs{s}")
            for s in range(n_streams)
        ]
        for s in range(n_streams):
            nc.sync.dma_start(out=S[s], in_=z0_d[:, :])
            nc.vector.memset(C[s], 0.0)

        # x buffers: [XP, GROUP*bs] ping-pong per stream. Rows IB:XP are
        # zeroed once; their weight rows are zero too, but fp16 garbage could
        # be NaN and NaN*0 would poison the PE accumulation.
        xbuf = [
            [
                singles.tile([XP, GROUP * bs], F16, name=f"xb{s}_{p}")
                for p in range(2)
            ]
            for s in range(n_streams)
        ]
        for s in range(n_streams):
            for p in range(2):
                nc.vector.memset(xbuf[s][p], 0.0)

        for t in range(n_rounds):
            if t % GROUP == 0:
                for s in range(n_streams):
                    buf = xbuf[s][(t // GROUP) % 2]
                    src = x_d[
                        IB * t : IB * (t + GROUP), s * bs : (s + 1) * bs
                    ].rearrange("(g i) b -> i g b", i=IB)
                    dst = buf[0:IB, :].rearrange("i (g b) -> i g b", g=GROUP)
                    nc.sync.dma_start(out=dst, in_=src)

            for s in range(n_streams):
                Sh, Cs = S[s], C[s]
                off = (t % GROUP) * bs
                xs = xbuf[s][(t // GROUP) % 2][:, off : off + bs]

                # Pre-activations: i,f,o share one 3-bank PSUM tensor; g gets
                # its own bank. Weight blocks in wh/wx: 0=i, 1=f, 2=g, 3=o.
                # g is computed first so tanh(g) overlaps the i/f/o matmuls,
                # shortening the recurrence critical path.
                Gifo = gp.tile([128, 3 * bs], F32, tag=f"Gifo{s}", name=f"Gifo{s}")
                Gg = gp.tile([128, bs], F32, tag=f"Gg{s}", name=f"Gg{s}")

                def mm(out, g):
                    # x-projection first: it only needs the x DMA and the
                    # PSUM bank (freed by last round's activation read), so it
                    # runs during the previous round's elementwise tail. Only
                    # the state matmul (needs h) sits on the recurrence chain.
                    cw = slice(128 * g, 128 * (g + 1))
                    nc.tensor.matmul(out, wx[:, cw], xs, start=True, stop=False)
                    nc.tensor.matmul(out, wh[:, cw], Sh, start=False, stop=True)

                # bf16 outputs: the ACT engine pays ~95ns extra per
                # instruction writing fp16 from a PSUM source; bf16 avoids it
                # and the gates are in [-1,1] where bf16 precision is plenty.
                Sifo = sp.tile([128, 3 * bs], BF16, tag=f"Sifo{s}", name=f"Sifo{s}")
                Sg = sp.tile([128, bs], BF16, tag=f"Sg{s}", name=f"Sg{s}")

                mm(Gg, 2)                                   # g
                nc.scalar.activation(Sg, Gg, AF.Tanh)
                mm(Gifo[:, 0:bs], 0)                        # i
                mm(Gifo[:, bs : 2 * bs], 1)                 # f
                nc.scalar.activation(
                    Sifo[:, 0 : 2 * bs], Gifo[:, 0 : 2 * bs], AF.Sigmoid
                )
                mm(Gifo[:, 2 * bs : 3 * bs], 3)             # o
                nc.scalar.activation(
                    Sifo[:, 2 * bs : 3 * bs], Gifo[:, 2 * bs : 3 * bs], AF.Sigmoid
                )

                t1 = sp.tile([128, bs], F16, tag=f"t1s{s}")
                t2 = sp.tile([128, bs], F16, tag=f"t2s{s}")
                nc.vector.tensor_mul(t1, Sifo[:, 0:bs], Sg)          # i * tanh(g)
                nc.vector.tensor_mul(t2, Sifo[:, bs : 2 * bs], Cs)   # f * c
                nc.vector.tensor_add(Cs, t1, t2)                     # c'
                tch = sp.tile([128, bs], F16, tag=f"tc{s}")
                nc.scalar.activation(tch, Cs, AF.Tanh)
                nc.vector.tensor_mul(Sh, Sifo[:, 2 * bs : 3 * bs], tch)  # h = o*tanh

            if t == 0:
                # Round 0 ran layer 2 on garbage (wavefront warmup); restore
                # its zero initial state. Layer 1's round-0 output is real.
                for s in range(n_streams):
                    nc.sync.dma_start(out=S[s][H:128, :], in_=z0_d[H:128, :])
                    nc.vector.memset(C[s][H:128, :], 0.0)

        # --- FC head on h2(T-1) = S[s][64:128] ---
        out_sb = singles.tile([2, b_core], F32)
        for s in range(n_streams):
            zp = gp.tile([H, bs], F32, tag=f"Gg{s}", name=f"zp{s}")
            nc.tensor.matmul(zp, wfc1, S[s], start=True, stop=True)
            zs = sp.tile([H + 1, bs], F16, tag=f"Sifo{s}", name=f"zs{s}")
            nc.vector.memset(zs[H : H + 1, :], 1.0)
            nc.scalar.activation(zs[0:H, :], zp, AF.Relu, bias=bfc1)
            op = gp.tile([2, bs], F32, tag=f"Gifo{s}", name=f"op{s}")
            nc.tensor.matmul(op, wfc2, zs[0 : H + 1, :], start=True, stop=True)
            nc.vector.tensor_copy(out_sb[:, s * bs : (s + 1) * bs], op)
        nc.sync.dma_start(out=y_d.rearrange("b c -> c b"), in_=out_sb)

    return nc


def prep_inputs(x, w_ih0, w_hh0, b_ih0, b_hh0, w_ih1, w_hh1, b_ih1, b_hh1,
                w_fc1, b_fc1, w_fc2, b_fc2, t_steps=T, bs=BS,
                n_streams=N_STREAMS, n_cores=N_CORES):
    """Host-side, layout-only prep. Returns per-core input maps."""
    f32 = np.float32
    f16 = np.float16
    x = np.asarray(x, f32)
    b_total, t_total, i_dim = x.shape
    ib = i_dim + 1
    b_core = bs * n_streams
    n_rounds = t_steps + 1
    n_groups = (n_rounds + GROUP - 1) // GROUP
    xt_rows = ib * n_groups * GROUP

    # [T, I+1, B]: per-step x^T plus a ones row (PE-side bias add). The ones
    # row stays 1.0 in the padding rounds so layer 2's bias is applied on the
    # final wavefront round.
    xt = np.ones((xt_rows // ib, ib, b_total), f16)
    xt[:, :i_dim, :] = 0.0
    xt[:t_total, :i_dim, :] = np.transpose(x, (1, 2, 0)).astype(f16)
    xt = np.ascontiguousarray(xt).reshape(xt_rows, b_total)

    w_hh0 = np.asarray(w_hh0, f32)
    w_ih0 = np.asarray(w_ih0, f32)
    w_ih1 = np.asarray(w_ih1, f32)
    w_hh1 = np.asarray(w_hh1, f32)
    b0 = np.asarray(b_ih0, f32) + np.asarray(b_hh0, f32)
    b1 = np.asarray(b_ih1, f32) + np.asarray(b_hh1, f32)
    wh = np.zeros((128, 512), f32)
    wx = np.zeros((XP, 512), f32)
    for g in range(4):
        rows = slice(H * g, H * (g + 1))
        wh[0:H, 128 * g : 128 * g + H] = w_hh0[rows, :].T
        wh[0:H, 128 * g + H : 128 * g + 128] = w_ih1[rows, :].T
        wh[H:128, 128 * g + H : 128 * g + 128] = w_hh1[rows, :].T
        wx[:i_dim, 128 * g : 128 * g + H] = w_ih0[rows, :].T
        wx[i_dim, 128 * g : 128 * g + H] = b0[rows]
        wx[i_dim, 128 * g + H : 128 * g + 128] = b1[rows]

    wfc1p = np.zeros((128, H), f32)
    wfc1p[H:128, :] = np.asarray(w_fc1, f32).T

    shared = dict(
        wh=wh.astype(f16),
        wx=wx.astype(f16),
        wfc1p=wfc1p.astype(f16),
        z0=np.zeros((128, bs), f16),
        bfc1=np.ascontiguousarray(np.asarray(b_fc1, f32)[:, None]),
        wfc2a=np.ascontiguousarray(
            np.concatenate(
                [np.asarray(w_fc2, f32).T, np.asarray(b_fc2, f32)[None, :]], 0
            ).astype(f16)
        ),
    )
    in_maps = []
    for k in range(n_cores):
        cols = slice(k * b_core, (k + 1) * b_core)
        m = dict(shared)
        m["xt"] = np.ascontiguousarray(xt[:, cols])
        in_maps.append(m)
    return in_maps


def kernel(**inputs):
    global LAST_RESULTS
    nc = bass.Bass()
    build(nc)
    in_maps = prep_inputs(**inputs)
    trace = bool(int(os.environ.get("LSTM_TRACE", "0")))
    res = run_bass_kernel_spmd(nc, in_maps, list(range(N_CORES)), trace=trace)
    LAST_RESULTS = res
    out = np.concatenate([res.results[k]["y"] for k in range(N_CORES)], axis=0)
    return out



# revision 22
# speedup vs baseline: 1.0328x; 1.0328x over previous
"""Trainium2 Bass kernel for a 2-layer LSTM (H=64, T=256) + FC head.

Strategy: data-parallel over batch (8192 -> 1024 per core x 8 cores).
Within a core, hidden-major layout: states are [128, 512] tiles with
layer-1 state on partitions 0-63 and layer-2 state on partitions 64-127,
batch on the free dim (2 streams of 512 for pipelining). The two LSTM
layers run as a wavefront (layer 2 lags layer 1 by one timestep), so a
single set of [128, 512] elementwise ops covers both layers each round.

Per gate, the pre-activations of both layers come from one K=128 matmul
against the packed state [h1; h2] (the L1 half of the weight matrix has
zeros over the h2 rows) plus one K=14 accumulating matmul for the x
contribution. All matmuls run the full 128-wide array at partition base
0 -- explicit PE tile_position quadrants crash this toolchain.

Host-side prep is layout-only: x is transposed to [T*14, B] so the
kernel can DMA contiguous [14, batch] slices each timestep.
"""

import os
from contextlib import ExitStack

import numpy as np

import concourse.bass as bass
import concourse.tile as tile
from concourse import mybir
from concourse.bass import MemorySpace
from concourse.bass_utils import run_bass_kernel_spmd

F32 = mybir.dt.float32
F16 = mybir.dt.float16
BF16 = mybir.dt.bfloat16
AF = mybir.ActivationFunctionType


def _wait_cap(inst) -> int:
    """Max sync waits the walrus in this image encodes per instruction."""
    if isinstance(inst, mybir.InstEventSemaphore):
        return 2
    return 1


def _patch_tile():
    """The walrus in this image encodes at most one sync wait on most
    instruction structs, but the Tile scheduler attaches several. Split the
    extras onto wait-only NOPs inserted just before the instruction (same
    engine), both during lowering and at the kernel-tail drain."""
    import bass_rust

    if getattr(tile.TileContext, "_ant_wait_patched", False):
        return

    def _split_waits(self, inst):
        si = inst.sync_info
        if si is None:
            return
        cap = _wait_cap(inst)
        waits = list(si.on_wait)
        if len(waits) <= cap:
            return
        si.on_wait = waits[: cap - 1] + waits[-1:] if cap > 1 else waits[-1:]
        extras = waits[cap - 1 : -1] if cap > 1 else waits[:-1]
        for w in extras:
            nop = mybir.InstNoOp(
                name=self.nc.get_next_instruction_name(), ins=[], outs=[]
            )
            nop.engine = inst.engine
            nop.sync_info = bass_rust.SyncInfo(on_wait=[w], on_update=[])
            self.nc.register_instruction(nop, overwrite=True)
            self.nc.cur_bb.bb.add_instruction(nop)

    orig_add = tile.TileContext._add_instruction

    def _add_instruction(self, inst):
        if inst.engine != mybir.EngineType.Unassigned:
            _split_waits(self, inst)
        return orig_add(self, inst)

    from concourse.vector_clock import ScopedClock

    def _drain_and_barrier(self, tick_clock, wait_clock):
        nc = self.nc
        drain_inst = nc.sync.drain()
        wait_clock.add_sem_waits(
            drain_inst.ins, ScopedClock({None: tick_clock.global_clock})
        )
        si = drain_inst.ins.sync_info
        if si is not None and len(si.on_wait) > 1:
            waits = list(si.on_wait)
            si.on_wait = waits[-1:]
            for w in waits[:-1]:
                nop = nc.sync.nop(nofuse=True, hint="drain_wait_split")
                nop.ins.sync_info = bass_rust.SyncInfo(on_wait=[w], on_update=[])

        nc.all_engine_barrier()
        assert self.sems is not None
        popped = nc._tile_sem_poison_stack.pop()
        assert popped is self._sem_poison
        nc.clear_and_free_semaphores(list(self.sems.allocated().values()))
        nc.all_engine_barrier()

    tile.TileContext._add_instruction = _add_instruction
    tile.TileContext._drain_and_barrier = _drain_and_barrier
    tile.TileContext._ant_wait_patched = True


_patch_tile()

N_CORES = 8
B, T, I, H = 8192, 256, 14, 64
IB = I + 1                     # x rows per step incl. the ones row (PE bias)
XP = 128                       # x-matmul K padded to 128: mixing PE row
                               # tile-sizes (K=128 vs K<128) between matmuls
                               # drops the PE to ~40% throughput, so every
                               # matmul uses K=128. Rows IB:XP carry zero
                               # weights so their rhs values never matter
B_CORE = B // N_CORES          # 1024
N_STREAMS = 2
BS = B_CORE // N_STREAMS       # 512 batch elems per stream
GROUP = 8                      # rounds per x-DMA group

LAST_RESULTS = None  # BassKernelResults of the most recent kernel() call


def build(nc, t_steps=T, bs=BS, n_streams=N_STREAMS):
    """Emit the per-core program. Identical on all cores (SPMD)."""
    n_rounds = t_steps + 1
    n_groups = (n_rounds + GROUP - 1) // GROUP
    xt_rows = IB * n_groups * GROUP
    b_core = bs * n_streams

    x_d = nc.dram_tensor("xt", (xt_rows, b_core), F16, kind="ExternalInput")
    wh_d = nc.dram_tensor("wh", (128, 512), F16, kind="ExternalInput")
    wx_d = nc.dram_tensor("wx", (XP, 512), F16, kind="ExternalInput")
    wfc1_d = nc.dram_tensor("wfc1p", (128, H), F16, kind="ExternalInput")
    bfc1_d = nc.dram_tensor("bfc1", (H, 1), F32, kind="ExternalInput")
    wfc2_d = nc.dram_tensor("wfc2a", (H + 1, 2), F16, kind="ExternalInput")
    z0_d = nc.dram_tensor("z0", (128, bs), F16, kind="ExternalInput")
    y_d = nc.dram_tensor("y", (b_core, 2), F32, kind="ExternalOutput")

    with tile.TileContext(nc) as tc, ExitStack() as ctx:
        singles = ctx.enter_context(tc.tile_pool(name="singles", bufs=1))
        gp = ctx.enter_context(
            tc.tile_pool(name="gates", bufs=1, space=MemorySpace.PSUM)
        )
        sp = ctx.enter_context(tc.tile_pool(name="work", bufs=2))

        wh = singles.tile([128, 512], F16)      # [[Whh0_g, Wih1_g],[0, Whh1_g]] x4
        nc.sync.dma_start(out=wh, in_=wh_d[:, :])
        wx = singles.tile([XP, 512], F16)       # [[Wih0_g | 0]; [b0_g | b1_g]] x4
        nc.sync.dma_start(out=wx, in_=wx_d[:, :])
        wfc1 = singles.tile([128, H], F16)      # [0; Wfc1^T]
        nc.sync.dma_start(out=wfc1, in_=wfc1_d[:, :])
        bfc1 = singles.tile([H, 1], F32)
        nc.sync.dma_start(out=bfc1, in_=bfc1_d[:, :])
        wfc2 = singles.tile([H + 1, 2], F16)
        nc.sync.dma_start(out=wfc2, in_=wfc2_d[:, :])

        # --- persistent state: [h1; h2] and [c1; c2] per stream ---
        S = [
            singles.tile([128, bs], F16, tag=f"Sh{s}", name=f"Sh{s}")
            for s in range(n_streams)
        ]
        C = [
            singles.tile([128, bs], F16, tag=f"Cs{s}", name=f"Cs{s}")
            for s in range(n_streams)
        ]
        for s in range(n_streams):
            nc.sync.dma_start(out=S[s], in_=z0_d[:, :])
            nc.vector.memset(C[s], 0.0)

        # x buffers: [XP, GROUP*bs] ping-pong per stream. Rows IB:XP are
        # zeroed once; their weight rows are zero too, but fp16 garbage could
        # be NaN and NaN*0 would poison the PE accumulation.
        xbuf = [
            [
                singles.tile([XP, GROUP * bs], F16, name=f"xb{s}_{p}")
                for p in range(2)
            ]
            for s in range(n_streams)
        ]
        for s in range(n_streams):
            for p in range(2):
                nc.vector.memset(xbuf[s][p], 0.0)

        for t in range(n_rounds):
            if t % GROUP == 0:
                for s in range(n_streams):
                    buf = xbuf[s][(t // GROUP) % 2]
                    src = x_d[
                        IB * t : IB * (t + GROUP), s * bs : (s + 1) * bs
                    ].rearrange("(g i) b -> i g b", i=IB)
                    dst = buf[0:IB, :].rearrange("i (g b) -> i g b", g=GROUP)
                    nc.sync.dma_start(out=dst, in_=src)

            for s in range(n_streams):
                Sh, Cs = S[s], C[s]
                off = (t % GROUP) * bs
                xs = xbuf[s][(t // GROUP) % 2][:, off : off + bs]

                # Pre-activations: i,f,o share one 3-bank PSUM tensor; g gets
                # its own bank. Weight blocks in wh/wx: 0=i, 1=f, 2=g, 3=o.
                # g is computed first so tanh(g) overlaps the i/f/o matmuls,
                # shortening the recurrence critical path.
                Gifo = gp.tile([128, 3 * bs], F32, tag=f"Gifo{s}", name=f"Gifo{s}")
                Gg = gp.tile([128, bs], F32, tag=f"Gg{s}", name=f"Gg{s}")

                # Engines execute in program order, so all four x-projection
                # matmuls come first: they only need the x DMA and their PSUM
                # bank (freed by last round's activation reads), and pre-run
                # during the previous round's elementwise tail. The state
                # matmuls (which wait on h) follow, so only they sit on the
                # recurrence chain.
                outs = [Gg, Gifo[:, 0:bs], Gifo[:, bs : 2 * bs],
                        Gifo[:, 2 * bs : 3 * bs]]
                blocks = (2, 0, 1, 3)                       # g, i, f, o
                for out, g in zip(outs, blocks):
                    cw = slice(128 * g, 128 * (g + 1))
                    nc.tensor.matmul(out, wx[:, cw], xs, start=True, stop=False)

                def hmm(k):
                    cw = slice(128 * blocks[k], 128 * (blocks[k] + 1))
                    nc.tensor.matmul(outs[k], wh[:, cw], Sh, start=False,
                                     stop=True)

                # bf16 outputs: the ACT engine pays ~95ns extra per
                # instruction writing fp16 from a PSUM source; bf16 avoids it
                # and the gates are in [-1,1] where bf16 precision is plenty.
                Sifo = sp.tile([128, 3 * bs], BF16, tag=f"Sifo{s}", name=f"Sifo{s}")
                Sg = sp.tile([128, bs], BF16, tag=f"Sg{s}", name=f"Sg{s}")

                hmm(0)                                      # g
                nc.scalar.activation(Sg, Gg, AF.Tanh)
                hmm(1)                                      # i
                hmm(2)                                      # f
                nc.scalar.activation(
                    Sifo[:, 0 : 2 * bs], Gifo[:, 0 : 2 * bs], AF.Sigmoid
                )
                hmm(3)                                      # o
                nc.scalar.activation(
                    Sifo[:, 2 * bs : 3 * bs], Gifo[:, 2 * bs : 3 * bs], AF.Sigmoid
                )

                t1 = sp.tile([128, bs], F16, tag=f"t1s{s}")
                t2 = sp.tile([128, bs], F16, tag=f"t2s{s}")
                nc.vector.tensor_mul(t1, Sifo[:, 0:bs], Sg)          # i * tanh(g)
                nc.vector.tensor_mul(t2, Sifo[:, bs : 2 * bs], Cs)   # f * c
                nc.vector.tensor_add(Cs, t1, t2)                     # c'
                tch = sp.tile([128, bs], F16, tag=f"tc{s}")
                nc.scalar.activation(tch, Cs, AF.Tanh)
                nc.vector.tensor_mul(Sh, Sifo[:, 2 * bs : 3 * bs], tch)  # h = o*tanh

            if t == 0:
                # Round 0 ran layer 2 on garbage (wavefront warmup); restore
                # its zero initial state. Layer 1's round-0 output is real.
                for s in range(n_streams):
                    nc.sync.dma_start(out=S[s][H:128, :], in_=z0_d[H:128, :])
                    nc.vector.memset(C[s][H:128, :], 0.0)

        # --- FC head on h2(T-1) = S[s][64:128] ---
        out_sb = singles.tile([2, b_core], F32)
        for s in range(n_streams):
            zp = gp.tile([H, bs], F32, tag=f"Gg{s}", name=f"zp{s}")
            nc.tensor.matmul(zp, wfc1, S[s], start=True, stop=True)
            zs = sp.tile([H + 1, bs], F16, tag=f"Sifo{s}", name=f"zs{s}")
            nc.vector.memset(zs[H : H + 1, :], 1.0)
            nc.scalar.activation(zs[0:H, :], zp, AF.Relu, bias=bfc1)
            op = gp.tile([2, bs], F32, tag=f"Gifo{s}", name=f"op{s}")
            nc.tensor.matmul(op, wfc2, zs[0 : H + 1, :], start=True, stop=True)
            nc.vector.tensor_copy(out_sb[:, s * bs : (s + 1) * bs], op)
        nc.sync.dma_start(out=y_d.rearrange("b c -> c b"), in_=out_sb)

    return nc


def prep_inputs(x, w_ih0, w_hh0, b_ih0, b_hh0, w_ih1, w_hh1, b_ih1, b_hh1,
                w_fc1, b_fc1, w_fc2, b_fc2, t_steps=T, bs=BS,
                n_streams=N_STREAMS, n_cores=N_CORES):
    """Host-side, layout-only prep. Returns per-core input maps."""
    f32 = np.float32
    f16 = np.float16
    x = np.asarray(x, f32)
    b_total, t_total, i_dim = x.shape
    ib = i_dim + 1
    b_core = bs * n_streams
    n_rounds = t_steps + 1
    n_groups = (n_rounds + GROUP - 1) // GROUP
    xt_rows = ib * n_groups * GROUP

    # [T, I+1, B]: per-step x^T plus a ones row (PE-side bias add). The ones
    # row stays 1.0 in the padding rounds so layer 2's bias is applied on the
    # final wavefront round.
    xt = np.ones((xt_rows // ib, ib, b_total), f16)
    xt[:, :i_dim, :] = 0.0
    xt[:t_total, :i_dim, :] = np.transpose(x, (1, 2, 0)).astype(f16)
    xt = np.ascontiguousarray(xt).reshape(xt_rows, b_total)

    w_hh0 = np.asarray(w_hh0, f32)
    w_ih0 = np.asarray(w_ih0, f32)
    w_ih1 = np.asarray(w_ih1, f32)
    w_hh1 = np.asarray(w_hh1, f32)
    b0 = np.asarray(b_ih0, f32) + np.asarray(b_hh0, f32)
    b1 = np.asarray(b_ih1, f32) + np.asarray(b_hh1, f32)
    wh = np.zeros((128, 512), f32)
    wx = np.zeros((XP, 512), f32)
    for g in range(4):
        rows = slice(H * g, H * (g + 1))
        wh[0:H, 128 * g : 128 * g + H] = w_hh0[rows, :].T
        wh[0:H, 128 * g + H : 128 * g + 128] = w_ih1[rows, :].T
        wh[H:128, 128 * g + H : 128 * g + 128] = w_hh1[rows, :].T
        wx[:i_dim, 128 * g : 128 * g + H] = w_ih0[rows, :].T
        wx[i_dim, 128 * g : 128 * g + H] = b0[rows]
        wx[i_dim, 128 * g + H : 128 * g + 128] = b1[rows]

    wfc1p = np.zeros((128, H), f32)
    wfc1p[H:128, :] = np.asarray(w_fc1, f32).T

    shared = dict(
        wh=wh.astype(f16),
        wx=wx.astype(f16),
        wfc1p=wfc1p.astype(f16),
        z0=np.zeros((128, bs), f16),
        bfc1=np.ascontiguousarray(np.asarray(b_fc1, f32)[:, None]),
        wfc2a=np.ascontiguousarray(
            np.concatenate(
                [np.asarray(w_fc2, f32).T, np.asarray(b_fc2, f32)[None, :]], 0
            ).astype(f16)
        ),
    )
    in_maps = []
    for k in range(n_cores):
        cols = slice(k * b_core, (k + 1) * b_core)
        m = dict(shared)
        m["xt"] = np.ascontiguousarray(xt[:, cols])
        in_maps.append(m)
    return in_maps


def kernel(**inputs):
    global LAST_RESULTS
    nc = bass.Bass()
    build(nc)
    in_maps = prep_inputs(**inputs)
    trace = bool(int(os.environ.get("LSTM_TRACE", "0")))
    res = run_bass_kernel_spmd(nc, in_maps, list(range(N_CORES)), trace=trace)
    LAST_RESULTS = res
    out = np.concatenate([res.results[k]["y"] for k in range(N_CORES)], axis=0)
    return out

